# revision 1
# baseline (speedup 1.0000x reference)
"""Trainium2 Bass kernel for LocalSpatialSimilarity.

Per sample (B=16, C=256, H=W=64, N=4096 pixels):
  s[p]  = sum_c x[c,p]                (channel sum)
  q[p]  = sum_c x[c,p]^2              (channel sum of squares)
  box   = 3x3 zero-padded box-sum of s (reshaped to 64x64)
  sim   = (box/9 * s) / sqrt(max(q * box^2 * 256/81, 1e-12))
  out   = softmax over p of (mask ? -inf : -sim)
        = (mask ? 0 : exp(-sim)) / total        (sim bounded in [-1,1] -> no
                                                 max-subtraction needed)

Sharding: pure data parallel, 2 samples per core across 8 cores.

On-chip layout: channels on partitions (two 128-chunks), pixels on the free
dim.  Channel reductions are ones-matmuls on the tensor engine into a
[8, 512] PSUM tile (stationary is an indicator column so block j of 512
pixels lands on psum partition j).  Spatial phase runs on a [64 rows,
2 samples, 64 cols] layout where the 3x3 box filter is partition-shifted /
free-shifted adds against a zero-padded tile.
"""

import sys

sys.path.insert(0, "/opt/trn_rl_repo")

import numpy as np

import concourse.bacc as bacc
import concourse.mybir as mybir
import concourse.tile as tile
from concourse.bass_utils import run_bass_kernel_spmd

B, C, H, W = 16, 256, 64, 64
N = H * W
NCORES = 8
SPC = B // NCORES  # samples per core
EPS2 = 1e-12
FP32 = mybir.dt.float32

# float32r: relaxed-precision fp32 matmul, 4x tensor-engine throughput.
MM_DT = mybir.dt.float32r

AF = mybir.ActivationFunctionType
ALU = mybir.AluOpType


def _kernel_body(ctx, tc, x, mask, vband, out, mm_dt=MM_DT, loop=1):
    nc = tc.nc
    HB = 2048  # pixels per spatial half

    consts = ctx.enter_context(tc.tile_pool(name="consts", bufs=1))
    xp = ctx.enter_context(tc.tile_pool(name="xp", bufs=4))
    sqp = ctx.enter_context(tc.tile_pool(name="sqp", bufs=3))
    rows = ctx.enter_context(tc.tile_pool(name="rows", bufs=4))
    single = ctx.enter_context(tc.tile_pool(name="single", bufs=2))
    psa = ctx.enter_context(tc.tile_pool(name="psa", bufs=4, space="PSUM"))
    pss = ctx.enter_context(tc.tile_pool(name="pss", bufs=1, space="PSUM"))

    # Stationary band: D[k, c] = 1 iff c == 7.  Slice [:, 7-j:15-j] is a
    # [128, 8] matrix whose only nonzero column is j, so the ones-matmul
    # lands block j's column sums on psum partition j (zeros elsewhere,
    # accumulated away).
    band = consts.tile([128, 15], FP32)
    nc.vector.memset(band[:], 0.0)
    nc.vector.memset(band[:, 7:8], 1.0)
    ones = consts.tile([128, 64], FP32)
    nc.vector.memset(ones[:], 1.0)
    # Tridiagonal 64x64 ones-band (host-provided): vertical 3-tap box sum as
    # a partition-space matmul (SBUF APs cannot start at unaligned
    # partitions, so partition-shifted adds are not expressible).
    band64 = consts.tile([64, 64], FP32)
    nc.sync.dma_start(out=band64[:], in_=vband.ap())

    for _it in range(loop):
        _one_pass(tc, x, mask, out, band, ones, band64, xp, sqp, rows, single, psa, pss)


def _one_pass(tc, x, mask, out, band, ones, band64, xp, sqp, rows, single, psa, pss):
    nc = tc.nc
    HB = 2048

    # Pair-batched spatial tiles: [row r, sample s, col c].
    Sb = single.tile([64, SPC, 64], FP32, tag="Sb")
    Qt = single.tile([64, SPC, 64], FP32, tag="Qt")

    # Mask, cast bool->f32 during DMA, then scaled to +1e30 ("-inf" additive).
    maskf = single.tile([64, SPC, 64], FP32, tag="maskf")
    nc.gpsimd.dma_start(out=maskf[:], in_=mask.ap().rearrange("s (r c) -> r s c", c=64))
    mb = single.tile([64, SPC, 64], FP32, tag="mb")
    nc.vector.tensor_scalar_mul(mb[:], maskf[:], 1e30)

    for s in range(SPC):
        ps_s = psa.tile([8, 512], FP32, tag="acc")
        ps_q = psa.tile([8, 512], FP32, tag="acc")
        # Whole channel-chunk loads: [128, 4096] with 16 KiB-contiguous rows,
        # alternating between the two HWDGE queues.
        x0 = xp.tile([128, N], FP32, tag="x")
        nc.sync.dma_start(out=x0[:], in_=x[s, 0:128, :])
        x1 = xp.tile([128, N], FP32, tag="x")
        nc.scalar.dma_start(out=x1[:], in_=x[s, 128:256, :])
        # Fold the two channel chunks before the matmul: halves PE work.
        # sf = x0 + x1 (DVE); squares in-place on ACT; qf = x0^2 + x1^2
        # folded in-place into x0's tile (DVE).
        sf = sqp.tile([128, N], FP32, tag="sf")
        nc.vector.tensor_add(sf[:], x0[:], x1[:])
        nc.scalar.activation(x0[:], x0[:], AF.Square)
        nc.scalar.activation(x1[:], x1[:], AF.Square)
        nc.vector.tensor_add(x0[:], x0[:], x1[:])
        for j in range(8):
            st = band[:, 7 - j : 15 - j]
            nc.tensor.matmul(
                ps_s[:],
                st,
                sf[:, 512 * j : 512 * (j + 1)],
                start=j == 0,
                stop=j == 7,
            )
            nc.tensor.matmul(
                ps_q[:],
                st,
                x0[:, 512 * j : 512 * (j + 1)],
                start=j == 0,
                stop=j == 7,
            )
        s_sb = rows.tile([8, 512], FP32, tag="srow")
        q_sb = rows.tile([8, 512], FP32, tag="qrow")
        nc.scalar.copy(s_sb[:], ps_s[:])
        nc.scalar.copy(q_sb[:], ps_q[:])
        # Reshape [8, 512] -> [64, 64]: both APs enumerate pixels in order.
        nc.sync.dma_start(out=Sb[:, s, :], in_=s_sb[:])
        nc.sync.dma_start(out=Qt[:, s, :], in_=q_sb[:])

    # 3x3 box-sum of S with zero padding: vertical 3-tap via tridiagonal
    # matmul over the row-partition dim, horizontal via free-shifted adds.
    v_ps = pss.tile([64, SPC * 64], FP32, tag="vps")
    nc.tensor.matmul(
        v_ps[:], band64[:], Sb[:].rearrange("r s c -> r (s c)"), start=True, stop=True
    )
    Hb = single.tile([64, SPC, 66], FP32)  # cols 0 and 65 stay zero
    nc.vector.memset(Hb[:], 0.0)
    nc.scalar.copy(Hb[:, :, 1:65], v_ps[:].rearrange("r (s c) -> r s c", c=64))
    T1 = single.tile([64, SPC, 64], FP32)
    nc.vector.tensor_add(T1[:], Hb[:, :, 0:64], Hb[:, :, 1:65])
    BOX = single.tile([64, SPC, 64], FP32)
    nc.vector.tensor_add(BOX[:], T1[:], Hb[:, :, 2:66])

    # D = max(box^2 * q * 256/81, eps^2);  R = D^-1/2 via exp(-0.5 ln D)
    # (Rsqrt activation is disallowed for accuracy reasons).
    P = single.tile([64, SPC, 64], FP32)
    nc.vector.tensor_mul(P[:], BOX[:], BOX[:])
    P2 = single.tile([64, SPC, 64], FP32)
    nc.vector.tensor_mul(P2[:], P[:], Qt[:])
    Dt = single.tile([64, SPC, 64], FP32)
    nc.vector.tensor_scalar(
        Dt[:], P2[:], 256.0 / 81.0, EPS2, op0=ALU.mult, op1=ALU.max
    )
    L = single.tile([64, SPC, 64], FP32)
    nc.scalar.activation(L[:], Dt[:], AF.Ln)
    R = single.tile([64, SPC, 64], FP32)
    nc.scalar.activation(R[:], L[:], AF.Exp, scale=-0.5)

    # U = box * s * R;  exp(-(U + 1e30*mask)/9) = masked exp(-sim)
    T = single.tile([64, SPC, 64], FP32)
    nc.vector.tensor_mul(T[:], BOX[:], Sb[:])
    U = single.tile([64, SPC, 64], FP32)
    nc.vector.tensor_mul(U[:], T[:], R[:])
    U2 = single.tile([64, SPC, 64], FP32)
    nc.vector.tensor_add(U2[:], U[:], mb[:])
    EM = single.tile([64, SPC, 64], FP32)
    rowsum = single.tile([64, SPC], FP32)
    for s in range(SPC):
        nc.scalar.activation(
            EM[:, s, :],
            U2[:, s, :],
            AF.Exp,
            scale=-1.0 / 9.0,
            accum_out=rowsum[:, s : s + 1],
        )

    # Per-sample totals: 64->1 ones-matmul, broadcast back 1->64, reciprocal.
    tot_ps = pss.tile([1, SPC], FP32, tag="tot")
    nc.tensor.matmul(tot_ps[:], ones[0:64, 0:1], rowsum[:], start=True, stop=True)
    tots = single.tile([1, SPC], FP32)
    nc.scalar.copy(tots[:], tot_ps[:])
    totb_ps = pss.tile([64, SPC], FP32, tag="totb")
    nc.tensor.matmul(totb_ps[:], ones[0:1, 0:64], tots[:], start=True, stop=True)
    rec = single.tile([64, SPC], FP32)
    nc.vector.reciprocal(rec[:], totb_ps[:])

    OUTt = single.tile([64, SPC, 64], FP32)
    for s in range(SPC):
        nc.vector.tensor_scalar_mul(
            OUTt[:, s, :], EM[:, s, :], rec[:, s : s + 1]
        )
    nc.sync.dma_start(
        out=out.ap().rearrange("s (r c) -> r s c", c=64), in_=OUTt[:]
    )


_NC_CACHE = {}


def _build(mm_dt=MM_DT, loop=1):
    key = (str(mm_dt), loop)
    if key in _NC_CACHE:
        return _NC_CACHE[key]
    nc = bacc.Bacc("TRN2", target_bir_lowering=False, debug=False)
    x = nc.declare_dram_parameter("x", [SPC, C, N], FP32, isOutput=False)
    mask = nc.declare_dram_parameter("mask", [SPC, N], mybir.dt.uint8, isOutput=False)
    vband = nc.declare_dram_parameter("vband", [64, 64], FP32, isOutput=False)
    out = nc.declare_dram_parameter("out", [SPC, N], FP32, isOutput=True)
    from contextlib import ExitStack

    with tile.TileContext(nc) as tc, ExitStack() as ctx:
        _kernel_body(ctx, tc, x, mask, vband, out, mm_dt, loop=loop)
    nc.compile()
    _NC_CACHE[key] = nc
    return nc


def band_matrix() -> np.ndarray:
    idx = np.arange(64)
    return (np.abs(idx[:, None] - idx[None, :]) <= 1).astype(np.float32)


def kernel(x: np.ndarray, prev_drop_mask: np.ndarray) -> np.ndarray:
    nc = _build()
    xs = np.ascontiguousarray(np.asarray(x), dtype=np.float32).reshape(B, C, N)
    ms = np.asarray(prev_drop_mask).astype(np.uint8).reshape(B, N)
    vb = band_matrix()
    in_maps = [
        {
            "x": xs[i * SPC : (i + 1) * SPC],
            "mask": ms[i * SPC : (i + 1) * SPC],
            "vband": vb,
        }
        for i in range(NCORES)
    ]
    res = run_bass_kernel_spmd(nc, in_maps, list(range(NCORES)))
    outs = [res.results[i]["out"] for i in range(NCORES)]
    return np.concatenate(outs, axis=0).reshape(B, H, W)



# revision 5
# speedup vs baseline: 1.5344x; 1.5344x over previous
"""Trainium2 Bass kernel for LocalSpatialSimilarity.

Per sample (B=16, C=256, H=W=64, N=4096 pixels):
  s[p]  = sum_c x[c,p]                (channel sum)
  q[p]  = sum_c x[c,p]^2              (channel sum of squares)
  box   = 3x3 zero-padded box-sum of s (reshaped to 64x64)
  D     = max(box^2 * q * 256/81, 1e-12)
  sim   = (box * s / 9) * rsqrt(D)
  out   = softmax over p of (mask ? -inf : -sim)
        = (mask ? 0 : exp(-sim)) / total        (sim bounded in [-1,1] -> no
                                                 max-subtraction needed)

Sharding: pure data parallel, 2 samples per core across 8 cores.

Pipeline layout: x streams in as ten [128, <=2048] pieces on the sync HWDGE
ring (kept free of all other traffic).  Per piece: channel-sum matmuls on the
tensor engine straight off the raw piece (float32r, one PE pass), squares on
alternating scalar/vector engines, then sum-of-squares matmuls.  The band
stationary trick lands 512-pixel blocks on psum partitions so each sample
accumulates into a single [8, 512] psum tile per quantity.

The spatial phase runs per sample (sample 0's tail hides under sample 1's DMA
stream): reshape [8,512]->[64,64] via an SBUF-to-SBUF DMA into a column-padded
tile, vertical 3-tap via a tridiagonal-stationary matmul, horizontal taps via
free-shifted adds, rsqrt via a Quake-style int-shift seed plus one
Newton-Raphson step on the vector engine (keeps every scalar-engine function
inside the exp_and_others activation-table set: exactly one table load).
"""

import sys

sys.path.insert(0, "/opt/trn_rl_repo")

import numpy as np

import concourse.bacc as bacc
import concourse.mybir as mybir
import concourse.tile as tile
from concourse.bass_utils import run_bass_kernel_spmd

B, C, H, W = 16, 256, 64, 64
N = H * W
NCORES = 8
SPC = B // NCORES  # samples per core
EPS2 = 1e-12
FP32 = mybir.dt.float32
I32 = mybir.dt.int32

# float32r: relaxed-precision fp32 matmul, single PE pass (plain fp32 = two).
MM_DT = mybir.dt.float32r
QUAKE_MAGIC = 0x5F3759DF
NR_ITERS = 1

AF = mybir.ActivationFunctionType
ALU = mybir.AluOpType

# x pieces streamed per (sample, channel-chunk): pixel (offset, length) lists.
# The final pieces shrink so the post-DMA tail (square + q-matmul) is short.
_PIECES = []
for _s in range(SPC):
    for _c in range(2):
        if _s == SPC - 1 and _c == 1:
            spans = [(0, 2048), (2048, 1024), (3072, 512), (3584, 512)]
        else:
            spans = [(0, 2048), (2048, 2048)]
        for _o, _l in spans:
            _PIECES.append((_s, _c, _o, _l))


def _kernel_body(ctx, tc, x, mask32, vband, hband, out):
    nc = tc.nc

    consts = ctx.enter_context(tc.tile_pool(name="consts", bufs=1))
    xp = ctx.enter_context(tc.tile_pool(name="xp", bufs=len(_PIECES)))
    sqp = ctx.enter_context(tc.tile_pool(name="sqp", bufs=3))
    rows = ctx.enter_context(tc.tile_pool(name="rows", bufs=4))
    sm = ctx.enter_context(tc.tile_pool(name="sm", bufs=2))
    psa = ctx.enter_context(tc.tile_pool(name="psa", bufs=2 * SPC, space="PSUM"))
    pss = ctx.enter_context(tc.tile_pool(name="pss", bufs=2, space="PSUM"))

    # ---- phase 0: input streams + constants ------------------------------
    # All x pieces go on the sync HWDGE ring, issued back-to-back with nothing
    # else queued on it, so the ring streams at full HBM rate.
    xts = []
    for s, c, o, ln in _PIECES:
        xt = xp.tile([128, ln], MM_DT, tag="x")
        nc.sync.dma_start(out=xt[:], in_=x[s, 128 * c : 128 * (c + 1), o : o + ln])
        xts.append(xt)

    # Small loads ride the scalar HWDGE ring.
    band64 = consts.tile([64, 64], FP32)
    nc.scalar.dma_start(out=band64[:], in_=vband.ap())
    maskt = consts.tile([64, SPC, 64], FP32)
    nc.scalar.dma_start(out=maskt[:], in_=mask32.ap())

    # Stationary band: D[k, c] = 1 iff c == 7.  Slice [:, 7-j:15-j] is a
    # [128, 8] matrix whose only nonzero column is j, so the ones-matmul
    # lands block j's column sums on psum partition j (zeros elsewhere,
    # accumulated away).  Host-provided: memset cannot write fp32r.
    band = consts.tile([128, 15], MM_DT)
    nc.scalar.dma_start(out=band[:], in_=hband.ap())
    ones64 = consts.tile([64, 64], FP32)
    nc.gpsimd.memset(ones64[:], 1.0)

    # Column-padded S tiles (cols 0 and 65 stay zero) and the shared rowsum.
    sbp = [consts.tile([64, 66], FP32, tag=f"sbp{s}", name=f"sbp{s}") for s in range(SPC)]
    for s in range(SPC):
        nc.gpsimd.memset(sbp[s][:, 0:1], 0.0)
        nc.gpsimd.memset(sbp[s][:, 65:66], 0.0)
    rowsum = consts.tile([64, SPC], FP32)

    ps_s = [psa.tile([8, 512], FP32, tag="acc", name=f"ps_s{i}") for i in range(SPC)]
    ps_q = [psa.tile([8, 512], FP32, tag="acc", name=f"ps_q{i}") for i in range(SPC)]

    # ---- phase A: streamed channel reductions ----------------------------
    nmm = [0] * SPC  # per-sample matmul counter (16 each for s and q)
    for k, (s, c, o, ln) in enumerate(_PIECES):
        xt = xts[k]
        first, last = nmm[s] == 0, nmm[s] + ln // 512 == 16
        for b in range(ln // 512):
            j = (o + 512 * b) // 512
            st = band[:, 7 - j : 15 - j]
            nc.tensor.matmul(
                ps_s[s][:],
                st,
                xt[:, 512 * b : 512 * (b + 1)],
                start=first and b == 0,
                stop=last and b == ln // 512 - 1,
            )
        sq = sqp.tile([128, ln], MM_DT, tag="sq")
        xf = xt[:].bitcast(FP32)
        if k % 2 == 0:
            nc.scalar.activation(sq[:], xf, AF.Square)
        else:
            nc.vector.tensor_mul(sq[:], xf, xf)
        for b in range(ln // 512):
            j = (o + 512 * b) // 512
            st = band[:, 7 - j : 15 - j]
            nc.tensor.matmul(
                ps_q[s][:],
                st,
                sq[:, 512 * b : 512 * (b + 1)],
                start=first and b == 0,
                stop=last and b == ln // 512 - 1,
            )
        nmm[s] += ln // 512
        if nmm[s] == 16:
            _spatial(tc, s, ps_s[s], ps_q[s], band64, ones64, sbp[s], maskt,
                     rowsum, rows, sm, pss, out)


def _spatial(tc, s, ps_s, ps_q, band64, ones64, sbp, maskt, rowsum, rows, sm,
             pss, out):
    """Box filter + cosine-sim + masked softmax for one sample.

    Sample SPC-1 is the kernel tail, so its small DMAs use the scalar HWDGE
    ring (lower first-byte latency); earlier samples are latency-irrelevant
    and ride SWDGE to keep the scalar engine free."""
    nc = tc.nc
    tail = s == SPC - 1
    dmae = nc.scalar if tail else nc.gpsimd

    # PSUM -> SBUF, then reshape [8,512] -> [64,64] (both APs enumerate the
    # 4096 pixels in order).  The reshape lands in the padded S tile.
    s_sb = rows.tile([8, 512], FP32, tag="srow")
    nc.scalar.copy(s_sb[:], ps_s[:])
    q_sb = rows.tile([8, 512], FP32, tag="qrow")
    nc.vector.tensor_copy(q_sb[:], ps_q[:])
    nc.sync.dma_start(out=sbp[:, 1:65], in_=s_sb[:])
    Qt = sm.tile([64, 64], FP32, tag="Qt")
    dmae.dma_start(out=Qt[:], in_=q_sb[:])

    # Vertical 3-tap via tridiagonal stationary (pad columns stay zero
    # through the matmul), horizontal taps via free-shifted adds.
    v_ps = pss.tile([64, 66], FP32, tag="spat")
    nc.tensor.matmul(v_ps[:], band64[:], sbp[:], start=True, stop=True)
    Hb = sm.tile([64, 66], FP32, tag="Hb")
    nc.scalar.copy(Hb[:], v_ps[:])
    T1 = sm.tile([64, 64], FP32, tag="T1")
    nc.vector.tensor_add(T1[:], Hb[:, 0:64], Hb[:, 1:65])
    BOX = sm.tile([64, 64], FP32, tag="BOX")
    nc.vector.tensor_add(BOX[:], T1[:], Hb[:, 2:66])

    # D = max(box^2 * q * 256/81, eps^2)
    P = sm.tile([64, 64], FP32, tag="P")
    nc.vector.tensor_mul(P[:], BOX[:], BOX[:])
    P2 = sm.tile([64, 64], FP32, tag="P2")
    nc.vector.tensor_mul(P2[:], P[:], Qt[:])
    Dt = sm.tile([64, 64], FP32, tag="Dt")
    nc.vector.tensor_scalar(
        Dt[:], P2[:], 256.0 / 81.0, EPS2, op0=ALU.mult, op1=ALU.max
    )

    # rsqrt(D) via the int-shift seed + NR (all on the vector engine; keeps
    # the scalar engine inside one activation-table set).
    ti = sm.tile([64, 64], I32, tag="ti")
    nc.vector.tensor_scalar(
        ti[:], Dt[:].bitcast(I32), 1, None, op0=ALU.logical_shift_right
    )
    yi = sm.tile([64, 64], I32, tag="yi")
    nc.vector.tensor_scalar(
        yi[:], ti[:], -1, QUAKE_MAGIC, op0=ALU.mult, op1=ALU.add
    )
    y = yi[:].bitcast(FP32)
    for _ in range(NR_ITERS):
        a = sm.tile([64, 64], FP32, tag="nra")
        nc.vector.tensor_mul(a[:], y, y)
        bb = sm.tile([64, 64], FP32, tag="nrb")
        nc.vector.tensor_mul(bb[:], a[:], Dt[:])
        cf = sm.tile([64, 64], FP32, tag="nrc")
        nc.vector.tensor_scalar(cf[:], bb[:], -0.5, 1.5, op0=ALU.mult, op1=ALU.add)
        yn = sm.tile([64, 64], FP32, tag="nry")
        nc.vector.tensor_mul(yn[:], y, cf[:])
        y = yn[:]

    # U = box * s * rsqrt;  exp(-(U + 1e30*mask)/9) = masked exp(-sim)
    T = sm.tile([64, 64], FP32, tag="T")
    nc.vector.tensor_mul(T[:], BOX[:], sbp[:, 1:65])
    U = sm.tile([64, 64], FP32, tag="U")
    nc.vector.tensor_mul(U[:], T[:], y)
    U2 = sm.tile([64, 64], FP32, tag="U2")
    nc.vector.tensor_add(U2[:], U[:], maskt[:, s, :])
    EM = sm.tile([64, 64], FP32, tag="EM")
    nc.scalar.activation(
        EM[:], U2[:], AF.Exp, scale=-1.0 / 9.0, accum_out=rowsum[:, s : s + 1]
    )

    # Per-sample total broadcast to all 64 partitions in one ones-matmul.
    totb = pss.tile([64, 1], FP32, tag="spat")
    nc.tensor.matmul(totb[:], ones64[:], rowsum[:, s : s + 1], start=True, stop=True)
    rec = sm.tile([64, 1], FP32, tag="rec")
    nc.vector.reciprocal(rec[:], totb[:])
    OUTt = sm.tile([64, 64], FP32, tag="OUTt")
    nc.vector.tensor_scalar_mul(OUTt[:], EM[:], rec[:, 0:1])
    dmae.dma_start(out=out[:, s, :], in_=OUTt[:])


_NC_CACHE = {}


def _build():
    key = "v1"
    if key in _NC_CACHE:
        return _NC_CACHE[key]
    nc = bacc.Bacc("TRN2", target_bir_lowering=False, debug=False)
    x = nc.declare_dram_parameter("x", [SPC, C, N], MM_DT, isOutput=False)
    mask32 = nc.declare_dram_parameter("mask32", [64, SPC, 64], FP32, isOutput=False)
    vband = nc.declare_dram_parameter("vband", [64, 64], FP32, isOutput=False)
    hband = nc.declare_dram_parameter("hband", [128, 15], MM_DT, isOutput=False)
    out = nc.declare_dram_parameter("out", [64, SPC, 64], FP32, isOutput=True)
    from contextlib import ExitStack

    with tile.TileContext(nc) as tc, ExitStack() as ctx:
        _kernel_body(ctx, tc, x, mask32, vband, hband, out)
    nc.compile()
    _NC_CACHE[key] = nc
    return nc


def band_matrix() -> np.ndarray:
    idx = np.arange(64)
    return (np.abs(idx[:, None] - idx[None, :]) <= 1).astype(np.float32)


def ind_band() -> np.ndarray:
    b = np.zeros((128, 15), dtype=np.float32)
    b[:, 7] = 1.0
    return b


def make_in_maps(x: np.ndarray, prev_drop_mask: np.ndarray) -> list:
    xs = np.ascontiguousarray(np.asarray(x), dtype=np.float32).reshape(B, C, N)
    # [B, N] bool -> per-core [64 rows, SPC, 64 cols] f32 pre-scaled +1e30.
    m32 = (np.asarray(prev_drop_mask).astype(np.float32) * 1e30).reshape(B, H, W)
    vb = band_matrix()
    hb = ind_band()
    maps = []
    for i in range(NCORES):
        mc = np.ascontiguousarray(m32[i * SPC : (i + 1) * SPC].transpose(1, 0, 2))
        maps.append(
            {
                "x": xs[i * SPC : (i + 1) * SPC],
                "mask32": mc,
                "vband": vb,
                "hband": hb,
            }
        )
    return maps


def gather_out(results) -> np.ndarray:
    # Each core returns [64, SPC, 64]; reorder to [B, H, W].
    outs = [
        np.asarray(results[i]["out"]).transpose(1, 0, 2) for i in range(NCORES)
    ]
    return np.concatenate(outs, axis=0).reshape(B, H, W)


def kernel(x: np.ndarray, prev_drop_mask: np.ndarray) -> np.ndarray:
    nc = _build()
    res = run_bass_kernel_spmd(nc, make_in_maps(x, prev_drop_mask), list(range(NCORES)))
    return gather_out(res.results)
